# revision 41
# baseline (speedup 1.0000x reference)
"""Quantized Linear (8-bit act / 4-bit weight fake-quant) on 8 Trainium2 cores.

Math (per reference):
  xq = rne(x / s_x) * s_x          s_x = max(absmax(x)/127, 1e-8)
  wq = rne(w / s_w) * s_w          s_w = max(absmax(w)/7,   1e-8)
  bq = rne(b / s_b) * s_b          s_b = max(absmax(b)/127, 1e-8)
  out_pre = bq + xq @ wq.T
  out = rne(out_pre / s_o) * s_o   s_o = max(absmax(out_pre)/127, 1e-8)

v16 design (2 token-groups x 4 outf-groups over 8 cores), ~391us vs the
591us v6 baseline (479us re-measured). Matmul phase runs 99%+ PE-busy at
the P0-downclocked ~2.0GHz floor; remaining time is the absmax->scale
AllReduce prologue (pinned to ~95us by a ~75-80us CC-firmware startup
wall that no trigger timing can beat) and the out-scale mesh + requant
tail. The w re-read stream is gated on the PAIR-mesh readback (~80us,
after every core's absmax and mesh trigger are done) so the first w
tiles are resident when the scales land: matmul starts ~97.7us.
 - x cast to fp16 on host (halves x HBM traffic; integer quantization
   absorbs the rounding; measured rel-err 1.09e-2 < 2e-2, deterministic).
 - prologue: absmax slices read in 1-2MB chunks split across both HWDGE
   rings, landing INSIDE the qwT/qx ring slots (wabs/xabs; x-g0 then
   quantizes straight from xabs, never re-read). Chunked DVE reduces
   pipeline with the DMA. ONE combined [wmax,xmax] AllReduce instead of
   two serial meshes; a pairwise warmup mini-mesh absorbs the CC startup
   wall and aligns cores (removing it costs ~20us of mesh peer-wait).
 - bulk w/x re-read streams FIFO-gated behind a mesh-result readback on
   each ring: ungated prefetch steals shared HBM bandwidth from slower
   peers' absmax reads and inflates the mesh peer-wait (27us observed).
 - kt-outer / jt-inner matmul order into 8 PSUM banks: every quantized
   2-kt chunk immediately feeds 8 matmuls, so the PE chases the quantize
   chain without starving (99%+ busy window).
 - PSUM evictions (bank*s_xw + bq -> fp16 opre) alternate ACT/DVE per
   jt; out-absmax reductions chase the evictions on DVE. The last group's
   final 4 kt-rounds run pair-major so bank closes stagger ~2us apart and
   the eviction + out-absmax chain overlaps the matmul window (drain
   7.4us -> 3.5us; full jt-major serial chains would stall the PE).
 - tail: fp16-magic (1536) requant mostly on DVE (fp16 DVE ~4x ACT
   rate), results coalesced into 1MB fp16 DMAs through idle wst slots.
 - (tried and rejected: fp8 DoubleRow split-matmul (1.44x < 2x needed),
   remote_dma_broadcast scale exchange (delivery/slot mapping broken),
   jt-outer final group (serial bank chain stalls PE), 0.5MB absmax
   chunks (per-DMA fixed cost), all-DVE evictions (ACT idle while DVE
   drains).)
"""

import sys

sys.path.insert(0, "/opt/trn_rl_repo")

import numpy as np

import concourse.bass as bass
import concourse.mybir as mybir
import concourse.tile as tile
from concourse import bacc, bass_isa

F32 = mybir.dt.float32
F16 = mybir.dt.float16
BF16 = mybir.dt.bfloat16
AF = mybir.ActivationFunctionType
ALU = mybir.AluOpType
AX = mybir.AxisListType

MAGIC = 12582912.0  # 1.5 * 2**23: fp32 add rounds to nearest-even integer
EPS = 1e-8
INV_QA = float(np.float32(1.0) / np.float32(127.0))
INV_QW = float(np.float32(1.0) / np.float32(7.0))

P = 128


def build(n_cores=8, T=4096, K=4096, J=4096):
    """SPMD program; host rolls each core's columns so that the exclusive
    absmax sub-slices are always the local leading 512 columns."""
    NTG, NJG = 2, 4
    TS = T // NTG            # 2048 tokens per core
    JS = J // NJG            # 1024 out-features per core
    n_kp = K // P            # 32 k-tiles
    GT = 512                 # token group width
    n_g = TS // GT           # 4 token groups
    n_jt = JS // P           # 8 j-tiles
    SB = 2                   # k-tiles per chunk
    n_ch = n_kp // SB        # 16 chunks per 512-wide column group

    nc = bacc.Bacc(
        "TRN2", target_bir_lowering=False, debug=False, num_devices=n_cores,
        monotonic_sem_count=2,
    )

    xg_d = nc.dram_tensor("xg", [n_g, P, n_kp, GT], F16, kind="ExternalInput")
    wg_d = nc.dram_tensor("wg", [2, P, n_kp, 512], F32, kind="ExternalInput")
    b_d = nc.dram_tensor("b_full", [J], F32, kind="ExternalInput")
    bs_d = nc.dram_tensor("b_shard", [JS], F32, kind="ExternalInput")
    og_d = nc.dram_tensor("og", [n_jt // 2, P, 2, TS], F16, kind="ExternalOutput")
    cc1_in = nc.dram_tensor("cc1_in", [1, 2], F32)
    cc1_out = nc.dram_tensor("cc1_out", [1, 2], F32)
    ccp_in = nc.dram_tensor("ccp_in", [1, 1], F32)
    ccp_out = nc.dram_tensor("ccp_out", [1, 1], F32)
    cc2_in = nc.dram_tensor("cc2_in", [1, 1], F32)
    cc2_out = nc.dram_tensor("cc2_out", [1, 1], F32)
    groups = [list(range(n_cores))]
    rsem = nc.monotonic_semaphore(0)   # remote-recv counting (pinned num)
    lsem = nc.monotonic_semaphore(1)   # local send-complete (never waited)

    with tile.TileContext(nc) as tc:
        with (
            tc.tile_pool(name="const", bufs=1) as const,
            tc.tile_pool(name="scal", bufs=1) as scal,
            tc.tile_pool(name="wst", bufs=2) as wst,
            tc.tile_pool(name="xst", bufs=2) as xst,
            tc.tile_pool(name="mid", bufs=3) as midp,
            tc.tile_pool(name="wq", bufs=1) as wqp,
            tc.tile_pool(name="xq", bufs=2) as xqp,
            tc.tile_pool(name="op", bufs=1) as opp,
            tc.tile_pool(name="outst", bufs=2) as outst,
            tc.tile_pool(name="mm", bufs=8, space="PSUM") as mmps,
        ):
            magic_t = const.tile([P, 1], F32)
            nc.vector.memset(magic_t[:], MAGIC)
            nmagic_t = const.tile([P, 1], F32)
            nc.vector.memset(nmagic_t[:], -MAGIC)

            # ---- CC warmup: pairwise mini-mesh aligns core pairs early and
            # warms the CC engine so the tail out-mesh begins promptly ----
            nc.sync.dma_start(ccp_in[:], magic_t[:1, :])
            nc.gpsimd.collective_compute(
                "AllReduce", ALU.max,
                replica_groups=[[2 * i, 2 * i + 1] for i in range(n_cores // 2)],
                ins=[ccp_in[:]], outs=[ccp_out[:]],
            )

            # ---- tiny bias loads first (16KB; negligible ring delay) ----
            bfull = scal.tile([P, J // P], F32)
            nc.sync.dma_start(bfull[:], b_d.rearrange("(p a) -> p a", p=P))
            bsh = scal.tile([P, n_jt], F32)
            nc.sync.dma_start(bsh[:], bs_d.rearrange("(a p) -> p a", p=P))

            # ---------------- Phase A: exclusive-slice absmax ----------------
            # Big 2MB/1MB chunks (amortize per-DMA fixed cost), split across
            # both HWDGE rings. The landing tiles sit in the qwT / qx-slot-0
            # ring slots (zero extra SBUF); x-g0 later quantizes directly
            # from xabs, so x-g0 is never re-read.
            wabs = wqp.tile([P, n_kp, 512], F32, tag="wq", name="wabs")
            xabs = xqp.tile([P, n_kp, GT], F16, tag="qx", name="xabs")
            amw = scal.tile([P, 4], F32)
            amx = scal.tile([P, 4], F32)
            for q in range(4):
                weng = nc.scalar if q % 2 == 0 else nc.sync
                xeng = nc.sync if q % 2 == 0 else nc.scalar
                weng.dma_start(
                    wabs[:, q * 8 : (q + 1) * 8, :], wg_d[0, :, q * 8 : (q + 1) * 8, :]
                )
                nc.vector.tensor_reduce(
                    amw[:, q : q + 1], wabs[:, q * 8 : (q + 1) * 8, :],
                    axis=AX.XY, op=ALU.max, apply_absolute_value=True,
                )
                xeng.dma_start(
                    xabs[:, q * 8 : (q + 1) * 8, :], xg_d[0, :, q * 8 : (q + 1) * 8, :]
                )
                nc.vector.tensor_reduce(
                    amx[:, q : q + 1], xabs[:, q * 8 : (q + 1) * 8, :],
                    axis=AX.XY, op=ALU.max, apply_absolute_value=True,
                )

            m2 = scal.tile([P, 2], F32)
            nc.vector.tensor_reduce(m2[:, 0:1], amw[:], axis=AX.X, op=ALU.max)
            nc.vector.tensor_reduce(m2[:, 1:2], amx[:], axis=AX.X, op=ALU.max)
            g2 = scal.tile([P, 2], F32)
            nc.gpsimd.partition_all_reduce(
                g2[:], m2[:], channels=P, reduce_op=bass_isa.ReduceOp.max
            )
            # cc input via the gpsimd SWDGE queue: par -> dma -> trigger all
            # FIFO on one engine, no cross-ring interference
            nc.gpsimd.dma_start(cc1_in[:], g2[:1, :])
            nc.gpsimd.collective_compute(
                "AllReduce", ALU.max, replica_groups=groups,
                ins=[cc1_in[:]], outs=[cc1_out[:]],
            )

            # ---- gates: bulk streams on both rings wait for the mesh ----
            # All 8 cores race their absmax reads against shared HBM
            # bandwidth; any bulk prefetch issued before the scales steals
            # bandwidth from slower peers' absmax reads. FIFO-ordering the
            # bulk DMAs behind a mesh-result readback keeps the race fair.
            gg = scal.tile([P, 2], F32)
            nc.sync.dma_start(gg[:1, :], cc1_out[:])
            # w-stream gate: the PAIR-mesh result, which lands ~12us before
            # the combined mesh completes. By then every core's absmax is
            # long done and every combined-mesh trigger has fired, so the
            # early w stream steals no critical bandwidth — but the first w
            # tiles are already in SBUF when the scales arrive, so quantize
            # (and the PE) starts immediately instead of waiting ~4us for
            # the first post-gate 1MB DMA.
            gpp = scal.tile([P, 1], F32)
            nc.scalar.dma_start(gpp[:1, :], ccp_out[:])

            # w: 1MB tiles of 4 k-tiles, all on the scalar ring.
            WCH = 4                      # k-tiles per w DMA tile
            n_wch = n_kp // WCH          # 8 tiles per 512-col group
            wre = []
            for i in range(2 * n_wch):
                jc, ii = divmod(i, n_wch)
                wf = wst.tile([P, WCH, 512], F32, tag="wst", name=f"wr{i}")
                nc.scalar.dma_start(
                    wf[:], wg_d[jc, :, ii * WCH : (ii + 1) * WCH, :]
                )
                wre.append(wf)

            # x: g1..g3 in 0.5MB tiles of 4 k-tiles on the sync ring
            # (g0 quantizes straight from xabs).
            xre = {}
            for g in range(1, n_g):
                for i in range(n_wch):
                    xf = xst.tile([P, WCH, GT], F16, tag="xst", name=f"xr{g}_{i}")
                    nc.sync.dma_start(xf[:], xg_d[g, :, i * WCH : (i + 1) * WCH, :])
                    xre[(g, i)] = xf

            bc2 = scal.tile([P, 2], F32)
            nc.gpsimd.partition_broadcast(bc2[:], gg[:1, :], channels=P)
            s_w = scal.tile([P, 1], F32)
            nc.vector.tensor_scalar(s_w[:], bc2[:, 0:1], INV_QW, EPS, op0=ALU.mult, op1=ALU.max)
            inv_sw = scal.tile([P, 1], F32)
            nc.vector.reciprocal(inv_sw[:], s_w[:])
            s_x = scal.tile([P, 1], F32)
            nc.vector.tensor_scalar(s_x[:], bc2[:, 1:2], INV_QA, EPS, op0=ALU.mult, op1=ALU.max)
            inv_sx = scal.tile([P, 1], F32)
            nc.vector.reciprocal(inv_sx[:], s_x[:])
            s_xw = scal.tile([P, 1], F32)
            nc.vector.tensor_tensor(out=s_xw[:], in0=s_x[:], in1=s_w[:], op=ALU.mult)

            # ---------------- Quantize helpers (ACT/DVE alternating) ---------
            def quant_chain(src_ap, dst_ap, inv_s, parity):
                mid = midp.tile([P, SB, 512], F32, tag="mid")
                if parity == 0:
                    nc.scalar.activation(
                        mid[:], src_ap, AF.Identity, bias=magic_t[:], scale=inv_s[:]
                    )
                    nc.vector.tensor_scalar(dst_ap, mid[:], -MAGIC, None, op0=ALU.add)
                else:
                    nc.vector.tensor_scalar(
                        mid[:], src_ap, inv_s[:], MAGIC, op0=ALU.mult, op1=ALU.add
                    )
                    nc.scalar.activation(
                        dst_ap, mid[:], AF.Identity, bias=nmagic_t[:], scale=1.0
                    )

            # ---------------- W/X prep + matmul ------------------------------
            qwT = wqp.tile([P, n_kp, JS], BF16, tag="wq", name="qwT")

            def wsrc(i):  # i-th [P,SB,512] w chunk (0..31 across jc0,jc1)
                return wre[i // 2][:, (i % 2) * SB : (i % 2 + 1) * SB, :]

            def xsrc(g, i):  # i-th [P,SB,GT] x chunk of group g
                if g == 0:
                    return xabs[:, i * SB : (i + 1) * SB, :]
                return xre[(g, i // 2)][:, (i % 2) * SB : (i % 2 + 1) * SB, :]

            def wprep_jc(jc):
                for i in range(n_ch):
                    k0 = i * SB
                    quant_chain(
                        wsrc(jc * n_ch + i),
                        qwT[:, k0 : k0 + SB, jc * 512 : (jc + 1) * 512],
                        inv_sw, i % 2,
                    )

            def prep_group(g, qx=None):
                if qx is None:
                    qx = xqp.tile([P, n_kp, GT], BF16, tag="qx", name=f"qx_{g}")
                for i in range(n_ch):
                    k0 = i * SB
                    quant_chain(
                        xsrc(g, i), qx[:, k0 : k0 + SB, :], inv_sx, (i + 1) % 2
                    )
                return qx

            # jc0 + g0 interleaved so both streams advance together
            qx0 = xqp.tile([P, n_kp, GT], BF16, tag="qx", name="qx_0")
            for i in range(n_ch):
                k0 = i * SB
                quant_chain(
                    wsrc(i), qwT[:, k0 : k0 + SB, 0:512], inv_sw, i % 2
                )
                quant_chain(
                    xsrc(0, i), qx0[:, k0 : k0 + SB, :], inv_sx, (i + 1) % 2
                )

            # bias scale + quantized bias (local; needed by first eviction)
            bmax0 = scal.tile([P, 1], F32)
            nc.vector.tensor_reduce(
                bmax0[:], bfull[:], axis=AX.X, op=ALU.max,
                apply_absolute_value=True,
            )
            bmax = scal.tile([P, 1], F32)
            nc.gpsimd.partition_all_reduce(
                bmax[:], bmax0[:], channels=P, reduce_op=bass_isa.ReduceOp.max
            )
            s_b = scal.tile([P, 1], F32)
            nc.vector.tensor_scalar(s_b[:], bmax[:], INV_QA, EPS, op0=ALU.mult, op1=ALU.max)
            inv_sb = scal.tile([P, 1], F32)
            nc.vector.reciprocal(inv_sb[:], s_b[:])
            by = scal.tile([P, n_jt], F32)
            nc.scalar.activation(by[:], bsh[:], AF.Identity, bias=magic_t[:], scale=inv_sb[:])
            bq = scal.tile([P, n_jt], F32)
            nc.vector.tensor_scalar(bq[:], by[:], -MAGIC, s_b[:], op0=ALU.add, op1=ALU.mult)

            # rest of the quantize streams
            wprep_jc(1)
            qx1 = prep_group(1)

            opre = opp.tile([P, n_jt, TS], F16, tag="op", name="opre")
            omax = scal.tile([P, n_g * n_jt], F32)

            def mm_group(g, qx, stagger_tail=False):
                bks = []
                for jt in range(n_jt):
                    bks.append(mmps.tile([P, GT], F32, tag="mm", name=f"mm_{g}_{jt}"))

                def mm(kt, jt):
                    nc.tensor.matmul(
                        bks[jt][:],
                        lhsT=qwT[:, kt, jt * P : (jt + 1) * P],
                        rhs=qx[:, kt, :],
                        start=(kt == 0),
                        stop=(kt == n_kp - 1),
                    )

                n_head = n_kp - 4 if stagger_tail else n_kp
                for kt in range(n_head):
                    for jt in range(n_jt):
                        mm(kt, jt)
                if stagger_tail:
                    # last group only: run the final 4 kt-rounds pair-major so
                    # bank closes stagger ~2us apart and the per-jt eviction +
                    # out-absmax chain overlaps the matmul window instead of
                    # serializing ~7us after the final matmul. Pair-major
                    # keeps adjacent instructions on different banks (a full
                    # serial per-bank chain measurably stalls the PE).
                    for pair in range(n_jt // 2):
                        for kt in range(n_head, n_kp):
                            mm(kt, 2 * pair)
                            mm(kt, 2 * pair + 1)
                return bks

            def evict_group(g, bks):
                for jt in range(n_jt):
                    oc = opre[:, jt, g * GT : (g + 1) * GT]
                    if jt % 2 == 0:
                        nc.scalar.activation(
                            oc, bks[jt][:], AF.Identity,
                            bias=bq[:, jt : jt + 1], scale=s_xw[:],
                        )
                    else:
                        nc.vector.tensor_scalar(
                            oc, bks[jt][:], s_xw[:], bq[:, jt : jt + 1],
                            op0=ALU.mult, op1=ALU.add,
                        )
                    nc.vector.tensor_reduce(
                        omax[:, g * n_jt + jt : g * n_jt + jt + 1], oc,
                        axis=AX.X, op=ALU.max, apply_absolute_value=True,
                    )

            bks0 = mm_group(0, qx0)
            evict_group(0, bks0)
            qx2 = prep_group(2)
            qx3 = prep_group(3)
            bks1 = mm_group(1, qx1)
            evict_group(1, bks1)
            bks2 = mm_group(2, qx2)
            evict_group(2, bks2)
            bks3 = mm_group(3, qx3, stagger_tail=True)
            evict_group(3, bks3)

            # ---------------- Tail: global out absmax -> requantize ---------
            om1 = scal.tile([P, 1], F32)
            nc.vector.tensor_reduce(om1[:], omax[:], axis=AX.X, op=ALU.max)
            omr = scal.tile([P, 1], F32)
            nc.gpsimd.partition_all_reduce(
                omr[:], om1[:], channels=P, reduce_op=bass_isa.ReduceOp.max
            )
            nc.scalar.dma_start(cc2_in[:], omr[:1, :])
            nc.gpsimd.collective_compute(
                "AllReduce", ALU.max, replica_groups=groups,
                ins=[cc2_in[:]], outs=[cc2_out[:]],
            )
            go = scal.tile([P, 1], F32)
            nc.sync.dma_start(go[:1, :], cc2_out[:])
            bco = scal.tile([P, 1], F32)
            nc.gpsimd.partition_broadcast(bco[:], go[:1, :], channels=P)
            s_o = scal.tile([P, 1], F32)
            nc.vector.tensor_scalar(s_o[:], bco[:], INV_QA, EPS, op0=ALU.mult, op1=ALU.max)
            inv_so = scal.tile([P, 1], F32)
            nc.vector.reciprocal(inv_so[:], s_o[:])
            # fp16 magic: out/s_o is in [-127,127], so 1536+v rounds to the
            # integer grid exactly in fp16 (ulp=1 in [1024,2048)).
            M16 = 1536.0
            m16_t = scal.tile([P, 1], F32)
            nc.vector.memset(m16_t[:], M16)
            nbt = scal.tile([P, 1], F32)  # -M16 * s_o for the ACT-second path
            nc.vector.tensor_scalar(nbt[:], s_o[:], -M16, None, op0=ALU.mult)

            # requant mostly on DVE (fp16 DVE ~4x faster than ACT); ACT takes
            # two of the eight pass1 ops. Output coalesced to 1MB DMAs via
            # idle wst ring slots.
            for pj in range(n_jt // 2):
                res = wst.tile([P, 2, TS], F16, tag="wst", name=f"ores{pj}")
                for h in range(2):
                    jt = 2 * pj + h
                    src = opre[:, jt, :]
                    oy = midp.tile([P, TS], F16, tag="mid")
                    if jt in (0, 4):
                        nc.scalar.activation(
                            oy[:], src, AF.Identity, bias=m16_t[:], scale=inv_so[:]
                        )
                    else:
                        nc.vector.tensor_scalar(
                            oy[:], src, inv_so[:], M16, op0=ALU.mult, op1=ALU.add
                        )
                    nc.vector.tensor_scalar(
                        res[:, h, :], oy[:], -M16, s_o[:], op0=ALU.add, op1=ALU.mult
                    )
                if pj == n_jt // 2 - 1:
                    nc.sync.dma_start(og_d[pj, :, 0:1, :], res[:, 0:1, :])
                    nc.scalar.dma_start(og_d[pj, :, 1:2, :], res[:, 1:2, :])
                else:
                    eng = nc.sync if pj % 2 == 0 else nc.scalar
                    eng.dma_start(og_d[pj, :, :, :], res[:])

    nc.compile()
    return nc


def _tile_pmajor(a2d, n_groups, gw):
    """[K, n_groups*gw] -> [n_groups, 128, K//128, gw] partition-major."""
    K = a2d.shape[0]
    return np.ascontiguousarray(
        a2d.reshape(K // 128, 128, n_groups, gw).transpose(2, 1, 0, 3)
    )


def _run(nc, inputs, n_cores, T, K, J, trace=False):
    from concourse.bass_utils import run_bass_kernel_spmd

    NTG, NJG = 2, 4
    TS, JS = T // NTG, J // NJG
    x = np.ascontiguousarray(inputs["x"], dtype=np.float32)
    w = np.ascontiguousarray(inputs["weight"], dtype=np.float32)
    b = np.ascontiguousarray(inputs["b"], dtype=np.float32)
    xT = np.ascontiguousarray(x.T.astype(np.float16))
    wT = np.ascontiguousarray(w.T)
    in_maps = []
    for c in range(n_cores):
        tg, jgr = divmod(c, NJG)
        xs = xT[:, tg * TS : (tg + 1) * TS]
        ws = wT[:, jgr * JS : (jgr + 1) * JS]
        bs = b[jgr * JS : (jgr + 1) * JS]
        # roll so the exclusive absmax sub-slice is the leading 512 columns
        xrr = np.roll(xs, -jgr * 512, axis=1)
        wrr = np.roll(ws, -tg * 512, axis=1)
        in_maps.append(
            {
                "xg": _tile_pmajor(xrr, TS // 512, 512),
                "wg": _tile_pmajor(wrr, JS // 512, 512),
                "b_full": b,
                "b_shard": np.ascontiguousarray(np.roll(bs, -tg * 512)),
            }
        )
    res = run_bass_kernel_spmd(nc, in_maps, core_ids=list(range(n_cores)), trace=trace)
    out = np.empty((T, J), dtype=np.float32)
    for c in range(n_cores):
        tg, jgr = divmod(c, NJG)
        og = res.results[c]["og"]  # [n_jt//2, 128, 2, TS]
        o = og.transpose(0, 2, 1, 3).reshape(JS, TS).astype(np.float32)
        o = np.roll(o, tg * 512, axis=0)
        o = np.roll(o, jgr * 512, axis=1)
        out[tg * TS : (tg + 1) * TS, jgr * JS : (jgr + 1) * JS] = o.T
    return out, res


_NC_CACHE = {}


def kernel(**inputs) -> np.ndarray:
    n_cores, T, K, J = 8, 4096, 4096, 4096
    key = (n_cores, T, K, J)
    if key not in _NC_CACHE:
        _NC_CACHE[key] = build(n_cores, T, K, J)
    out, _ = _run(_NC_CACHE[key], inputs, n_cores, T, K, J)
    return out


# revision 44
# speedup vs baseline: 1.0616x; 1.0616x over previous
"""Quantized Linear (8-bit act / 4-bit weight fake-quant) on 8 Trainium2 cores.

Math (per reference):
  xq = rne(x / s_x) * s_x          s_x = max(absmax(x)/127, 1e-8)
  wq = rne(w / s_w) * s_w          s_w = max(absmax(w)/7,   1e-8)
  bq = rne(b / s_b) * s_b          s_b = max(absmax(b)/127, 1e-8)
  out_pre = bq + xq @ wq.T
  out = rne(out_pre / s_o) * s_o   s_o = max(absmax(out_pre)/127, 1e-8)

v16 design (2 token-groups x 4 outf-groups over 8 cores), ~391us vs the
591us v6 baseline (479us re-measured). Matmul phase runs 99%+ PE-busy at
the P0-downclocked ~2.0GHz floor; remaining time is the absmax->scale
AllReduce prologue (pinned to ~95us by a ~75-80us CC-firmware startup
wall that no trigger timing can beat) and the out-scale mesh + requant
tail. The w re-read stream is gated on the PAIR-mesh readback (~80us,
after every core's absmax and mesh trigger are done) so the first w
tiles are resident when the scales land: matmul starts ~97.7us.
 - x cast to fp16 on host (halves x HBM traffic; integer quantization
   absorbs the rounding; measured rel-err 1.09e-2 < 2e-2, deterministic).
 - prologue: absmax slices read in 1-2MB chunks split across both HWDGE
   rings, landing INSIDE the qwT/qx ring slots (wabs/xabs; x-g0 then
   quantizes straight from xabs, never re-read). Chunked DVE reduces
   pipeline with the DMA. ONE combined [wmax,xmax] AllReduce instead of
   two serial meshes; a pairwise warmup mini-mesh absorbs the CC startup
   wall and aligns cores (removing it costs ~20us of mesh peer-wait).
 - bulk w/x re-read streams FIFO-gated behind a mesh-result readback on
   each ring: ungated prefetch steals shared HBM bandwidth from slower
   peers' absmax reads and inflates the mesh peer-wait (27us observed).
 - kt-outer / jt-inner matmul order into 8 PSUM banks: every quantized
   2-kt chunk immediately feeds 8 matmuls, so the PE chases the quantize
   chain without starving (99%+ busy window).
 - PSUM evictions (bank*s_xw + bq -> fp16 opre) alternate ACT/DVE per
   jt; out-absmax reductions chase the evictions on DVE. The last group's
   final 4 kt-rounds run pair-major so bank closes stagger ~2us apart and
   the eviction + out-absmax chain overlaps the matmul window (drain
   7.4us -> 3.5us; full jt-major serial chains would stall the PE).
 - tail: fp16-magic (1536) requant mostly on DVE (fp16 DVE ~4x ACT
   rate), results coalesced into 1MB fp16 DMAs through idle wst slots.
 - (tried and rejected: fp8 DoubleRow split-matmul (1.44x < 2x needed),
   remote_dma_broadcast scale exchange (delivery/slot mapping broken),
   jt-outer final group (serial bank chain stalls PE), 0.5MB absmax
   chunks (per-DMA fixed cost), all-DVE evictions (ACT idle while DVE
   drains).)
"""

import sys

sys.path.insert(0, "/opt/trn_rl_repo")

import numpy as np

import concourse.bass as bass
import concourse.mybir as mybir
import concourse.tile as tile
from concourse import bacc, bass_isa

F32 = mybir.dt.float32
F16 = mybir.dt.float16
BF16 = mybir.dt.bfloat16
AF = mybir.ActivationFunctionType
ALU = mybir.AluOpType
AX = mybir.AxisListType

MAGIC = 12582912.0  # 1.5 * 2**23: fp32 add rounds to nearest-even integer
EPS = 1e-8
INV_QA = float(np.float32(1.0) / np.float32(127.0))
INV_QW = float(np.float32(1.0) / np.float32(7.0))

P = 128


def build(n_cores=8, T=4096, K=4096, J=4096):
    """SPMD program; host rolls each core's columns so that the exclusive
    absmax sub-slices are always the local leading 512 columns."""
    NTG, NJG = 2, 4
    TS = T // NTG            # 2048 tokens per core
    JS = J // NJG            # 1024 out-features per core
    n_kp = K // P            # 32 k-tiles
    GT = 512                 # token group width
    n_g = TS // GT           # 4 token groups
    n_jt = JS // P           # 8 j-tiles
    SB = 2                   # k-tiles per chunk
    n_ch = n_kp // SB        # 16 chunks per 512-wide column group

    nc = bacc.Bacc(
        "TRN2", target_bir_lowering=False, debug=False, num_devices=n_cores,
        monotonic_sem_count=2,
    )

    xg_d = nc.dram_tensor("xg", [n_g, P, n_kp, GT], F16, kind="ExternalInput")
    wg_d = nc.dram_tensor("wg", [2, P, n_kp, 512], F32, kind="ExternalInput")
    b_d = nc.dram_tensor("b_full", [J], F32, kind="ExternalInput")
    bs_d = nc.dram_tensor("b_shard", [JS], F32, kind="ExternalInput")
    og_d = nc.dram_tensor("og", [n_jt // 2, P, 2, TS], F16, kind="ExternalOutput")
    cc1_in = nc.dram_tensor("cc1_in", [1, 2], F32)
    cc1_out = nc.dram_tensor("cc1_out", [1, 2], F32)
    ccp_in = nc.dram_tensor("ccp_in", [1, 1], F32)
    ccp_out = nc.dram_tensor("ccp_out", [1, 1], F32)
    cc2_in = nc.dram_tensor("cc2_in", [1, 1], F32)
    cc2_out = nc.dram_tensor("cc2_out", [1, 1], F32)
    groups = [list(range(n_cores))]
    rsem = nc.monotonic_semaphore(0)   # remote-recv counting (pinned num)
    lsem = nc.monotonic_semaphore(1)   # local send-complete (never waited)

    with tile.TileContext(nc) as tc:
        with (
            tc.tile_pool(name="const", bufs=1) as const,
            tc.tile_pool(name="scal", bufs=1) as scal,
            tc.tile_pool(name="wst", bufs=2) as wst,
            tc.tile_pool(name="xst", bufs=2) as xst,
            tc.tile_pool(name="mid", bufs=3) as midp,
            tc.tile_pool(name="wq", bufs=1) as wqp,
            tc.tile_pool(name="xq", bufs=2) as xqp,
            tc.tile_pool(name="op", bufs=1) as opp,
            tc.tile_pool(name="outst", bufs=2) as outst,
            tc.tile_pool(name="mm", bufs=8, space="PSUM") as mmps,
        ):
            magic_t = const.tile([P, 1], F32)
            nc.vector.memset(magic_t[:], MAGIC)
            nmagic_t = const.tile([P, 1], F32)
            nc.vector.memset(nmagic_t[:], -MAGIC)

            # ---- CC warmup: pairwise mini-mesh aligns core pairs early and
            # warms the CC engine so the tail out-mesh begins promptly ----
            nc.sync.dma_start(ccp_in[:], magic_t[:1, :])
            nc.gpsimd.collective_compute(
                "AllReduce", ALU.max,
                replica_groups=[[2 * i, 2 * i + 1] for i in range(n_cores // 2)],
                ins=[ccp_in[:]], outs=[ccp_out[:]],
            )

            # ---- tiny bias loads first (16KB; negligible ring delay) ----
            bfull = scal.tile([P, J // P], F32)
            nc.sync.dma_start(bfull[:], b_d.rearrange("(p a) -> p a", p=P))
            bsh = scal.tile([P, n_jt], F32)
            nc.sync.dma_start(bsh[:], bs_d.rearrange("(a p) -> p a", p=P))

            # ---------------- Phase A: exclusive-slice absmax ----------------
            # Big 2MB/1MB chunks (amortize per-DMA fixed cost), split across
            # both HWDGE rings. The landing tiles sit in the qwT / qx-slot-0
            # ring slots (zero extra SBUF); x-g0 later quantizes directly
            # from xabs, so x-g0 is never re-read.
            wabs = wqp.tile([P, n_kp, 512], F32, tag="wq", name="wabs")
            xabs = xqp.tile([P, n_kp, GT], F16, tag="qx", name="xabs")
            amw = scal.tile([P, 4], F32)
            amx = scal.tile([P, 4], F32)
            for q in range(4):
                weng = nc.scalar if q % 2 == 0 else nc.sync
                xeng = nc.sync if q % 2 == 0 else nc.scalar
                weng.dma_start(
                    wabs[:, q * 8 : (q + 1) * 8, :], wg_d[0, :, q * 8 : (q + 1) * 8, :]
                )
                nc.vector.tensor_reduce(
                    amw[:, q : q + 1], wabs[:, q * 8 : (q + 1) * 8, :],
                    axis=AX.XY, op=ALU.max, apply_absolute_value=True,
                )
                xeng.dma_start(
                    xabs[:, q * 8 : (q + 1) * 8, :], xg_d[0, :, q * 8 : (q + 1) * 8, :]
                )
                nc.vector.tensor_reduce(
                    amx[:, q : q + 1], xabs[:, q * 8 : (q + 1) * 8, :],
                    axis=AX.XY, op=ALU.max, apply_absolute_value=True,
                )

            m2 = scal.tile([P, 2], F32)
            nc.vector.tensor_reduce(m2[:, 0:1], amw[:], axis=AX.X, op=ALU.max)
            nc.vector.tensor_reduce(m2[:, 1:2], amx[:], axis=AX.X, op=ALU.max)
            # pre-scale by [1/7, 1/127] BEFORE the mesh (max commutes with
            # positive scaling exactly), so the post-mesh critical path is
            # 3 fused [P,2] ops instead of 7. Column writes via tensor_scalar
            # (memset on a column slice silently fails).
            m2s = scal.tile([P, 2], F32)
            nc.vector.tensor_scalar(m2s[:, 0:1], m2[:, 0:1], INV_QW, None, op0=ALU.mult)
            nc.vector.tensor_scalar(m2s[:, 1:2], m2[:, 1:2], INV_QA, None, op0=ALU.mult)
            g2 = scal.tile([P, 2], F32)
            nc.gpsimd.partition_all_reduce(
                g2[:], m2s[:], channels=P, reduce_op=bass_isa.ReduceOp.max
            )
            # cc input via the gpsimd SWDGE queue: par -> dma -> trigger all
            # FIFO on one engine, no cross-ring interference
            nc.gpsimd.dma_start(cc1_in[:], g2[:1, :])
            nc.gpsimd.collective_compute(
                "AllReduce", ALU.max, replica_groups=groups,
                ins=[cc1_in[:]], outs=[cc1_out[:]],
            )

            # ---- gates: bulk streams on both rings wait for the mesh ----
            # All 8 cores race their absmax reads against shared HBM
            # bandwidth; any bulk prefetch issued before the scales steals
            # bandwidth from slower peers' absmax reads. FIFO-ordering the
            # bulk DMAs behind a mesh-result readback keeps the race fair.
            gg = scal.tile([P, 2], F32)
            nc.sync.dma_start(gg[:1, :], cc1_out[:])
            # w-stream gate: the PAIR-mesh result, which lands ~12us before
            # the combined mesh completes. By then every core's absmax is
            # long done and every combined-mesh trigger has fired, so the
            # early w stream steals no critical bandwidth — but the first w
            # tiles are already in SBUF when the scales arrive, so quantize
            # (and the PE) starts immediately instead of waiting ~4us for
            # the first post-gate 1MB DMA.
            gpp = scal.tile([P, 1], F32)
            nc.scalar.dma_start(gpp[:1, :], ccp_out[:])

            # w: 1MB tiles of 4 k-tiles, all on the scalar ring.
            WCH = 4                      # k-tiles per w DMA tile
            n_wch = n_kp // WCH          # 8 tiles per 512-col group
            wre = []
            for i in range(2 * n_wch):
                jc, ii = divmod(i, n_wch)
                wf = wst.tile([P, WCH, 512], F32, tag="wst", name=f"wr{i}")
                nc.scalar.dma_start(
                    wf[:], wg_d[jc, :, ii * WCH : (ii + 1) * WCH, :]
                )
                wre.append(wf)

            # x: g1..g3 in 0.5MB tiles of 4 k-tiles on the sync ring
            # (g0 quantizes straight from xabs).
            xre = {}
            for g in range(1, n_g):
                for i in range(n_wch):
                    xf = xst.tile([P, WCH, GT], F16, tag="xst", name=f"xr{g}_{i}")
                    nc.sync.dma_start(xf[:], xg_d[g, :, i * WCH : (i + 1) * WCH, :])
                    xre[(g, i)] = xf

            bc2 = scal.tile([P, 2], F32)
            nc.gpsimd.partition_broadcast(bc2[:], gg[:1, :], channels=P)
            s2 = scal.tile([P, 2], F32)
            nc.vector.tensor_scalar(s2[:], bc2[:], EPS, None, op0=ALU.max)
            inv2 = scal.tile([P, 2], F32)
            nc.vector.reciprocal(inv2[:], s2[:])
            inv_sw = inv2[:, 0:1]
            inv_sx = inv2[:, 1:2]
            s_xw = scal.tile([P, 1], F32)
            nc.vector.tensor_tensor(out=s_xw[:], in0=s2[:, 1:2], in1=s2[:, 0:1], op=ALU.mult)

            # ---------------- Quantize helpers (ACT/DVE alternating) ---------
            def quant_chain(src_ap, dst_ap, inv_s, parity):
                mid = midp.tile([P, SB, 512], F32, tag="mid")
                if parity == 0:
                    nc.scalar.activation(
                        mid[:], src_ap, AF.Identity, bias=magic_t[:], scale=inv_s
                    )
                    nc.vector.tensor_scalar(dst_ap, mid[:], -MAGIC, None, op0=ALU.add)
                else:
                    nc.vector.tensor_scalar(
                        mid[:], src_ap, inv_s, MAGIC, op0=ALU.mult, op1=ALU.add
                    )
                    nc.scalar.activation(
                        dst_ap, mid[:], AF.Identity, bias=nmagic_t[:], scale=1.0
                    )

            # ---------------- W/X prep + matmul ------------------------------
            qwT = wqp.tile([P, n_kp, JS], BF16, tag="wq", name="qwT")

            def wsrc(i):  # i-th [P,SB,512] w chunk (0..31 across jc0,jc1)
                return wre[i // 2][:, (i % 2) * SB : (i % 2 + 1) * SB, :]

            def xsrc(g, i):  # i-th [P,SB,GT] x chunk of group g
                if g == 0:
                    return xabs[:, i * SB : (i + 1) * SB, :]
                return xre[(g, i // 2)][:, (i % 2) * SB : (i % 2 + 1) * SB, :]

            def wprep_jc(jc):
                for i in range(n_ch):
                    k0 = i * SB
                    quant_chain(
                        wsrc(jc * n_ch + i),
                        qwT[:, k0 : k0 + SB, jc * 512 : (jc + 1) * 512],
                        inv_sw, i % 2,
                    )

            def prep_group(g, qx=None):
                if qx is None:
                    qx = xqp.tile([P, n_kp, GT], BF16, tag="qx", name=f"qx_{g}")
                for i in range(n_ch):
                    k0 = i * SB
                    quant_chain(
                        xsrc(g, i), qx[:, k0 : k0 + SB, :], inv_sx, (i + 1) % 2
                    )
                return qx

            # jc0 + g0 interleaved so both streams advance together
            qx0 = xqp.tile([P, n_kp, GT], BF16, tag="qx", name="qx_0")
            for i in range(n_ch):
                k0 = i * SB
                quant_chain(
                    wsrc(i), qwT[:, k0 : k0 + SB, 0:512], inv_sw, i % 2
                )
                quant_chain(
                    xsrc(0, i), qx0[:, k0 : k0 + SB, :], inv_sx, (i + 1) % 2
                )

            # bias scale + quantized bias (local; needed by first eviction)
            bmax0 = scal.tile([P, 1], F32)
            nc.vector.tensor_reduce(
                bmax0[:], bfull[:], axis=AX.X, op=ALU.max,
                apply_absolute_value=True,
            )
            bmax = scal.tile([P, 1], F32)
            nc.gpsimd.partition_all_reduce(
                bmax[:], bmax0[:], channels=P, reduce_op=bass_isa.ReduceOp.max
            )
            s_b = scal.tile([P, 1], F32)
            nc.vector.tensor_scalar(s_b[:], bmax[:], INV_QA, EPS, op0=ALU.mult, op1=ALU.max)
            inv_sb = scal.tile([P, 1], F32)
            nc.vector.reciprocal(inv_sb[:], s_b[:])
            by = scal.tile([P, n_jt], F32)
            nc.scalar.activation(by[:], bsh[:], AF.Identity, bias=magic_t[:], scale=inv_sb[:])
            bq = scal.tile([P, n_jt], F32)
            nc.vector.tensor_scalar(bq[:], by[:], -MAGIC, s_b[:], op0=ALU.add, op1=ALU.mult)

            # rest of the quantize streams
            wprep_jc(1)
            qx1 = prep_group(1)

            opre = opp.tile([P, n_jt, TS], F16, tag="op", name="opre")
            omax = scal.tile([P, n_g * n_jt], F32)

            def mm_group(g, qx, stagger_tail=False):
                bks = []
                for jt in range(n_jt):
                    bks.append(mmps.tile([P, GT], F32, tag="mm", name=f"mm_{g}_{jt}"))

                def mm(kt, jt):
                    nc.tensor.matmul(
                        bks[jt][:],
                        lhsT=qwT[:, kt, jt * P : (jt + 1) * P],
                        rhs=qx[:, kt, :],
                        start=(kt == 0),
                        stop=(kt == n_kp - 1),
                    )

                n_head = n_kp - 4 if stagger_tail else n_kp
                for kt in range(n_head):
                    for jt in range(n_jt):
                        mm(kt, jt)
                if stagger_tail:
                    # last group only: run the final 4 kt-rounds pair-major so
                    # bank closes stagger ~2us apart and the per-jt eviction +
                    # out-absmax chain overlaps the matmul window instead of
                    # serializing ~7us after the final matmul. Pair-major
                    # keeps adjacent instructions on different banks (a full
                    # serial per-bank chain measurably stalls the PE).
                    for pair in range(n_jt // 2):
                        for kt in range(n_head, n_kp):
                            mm(kt, 2 * pair)
                            mm(kt, 2 * pair + 1)
                return bks

            def evict_group(g, bks):
                for jt in range(n_jt):
                    oc = opre[:, jt, g * GT : (g + 1) * GT]
                    if jt % 2 == 0:
                        nc.scalar.activation(
                            oc, bks[jt][:], AF.Identity,
                            bias=bq[:, jt : jt + 1], scale=s_xw[:],
                        )
                    else:
                        nc.vector.tensor_scalar(
                            oc, bks[jt][:], s_xw[:], bq[:, jt : jt + 1],
                            op0=ALU.mult, op1=ALU.add,
                        )
                    nc.vector.tensor_reduce(
                        omax[:, g * n_jt + jt : g * n_jt + jt + 1], oc,
                        axis=AX.X, op=ALU.max, apply_absolute_value=True,
                    )

            bks0 = mm_group(0, qx0)
            evict_group(0, bks0)
            qx2 = prep_group(2)
            qx3 = prep_group(3)
            bks1 = mm_group(1, qx1)
            evict_group(1, bks1)
            bks2 = mm_group(2, qx2)
            evict_group(2, bks2)
            bks3 = mm_group(3, qx3, stagger_tail=True)
            evict_group(3, bks3)

            # ---------------- Tail: global out absmax -> requantize ---------
            om1 = scal.tile([P, 1], F32)
            nc.vector.tensor_reduce(om1[:], omax[:], axis=AX.X, op=ALU.max)
            omr = scal.tile([P, 1], F32)
            nc.gpsimd.partition_all_reduce(
                omr[:], om1[:], channels=P, reduce_op=bass_isa.ReduceOp.max
            )
            nc.scalar.dma_start(cc2_in[:], omr[:1, :])
            nc.gpsimd.collective_compute(
                "AllReduce", ALU.max, replica_groups=groups,
                ins=[cc2_in[:]], outs=[cc2_out[:]],
            )
            go = scal.tile([P, 1], F32)
            nc.sync.dma_start(go[:1, :], cc2_out[:])
            bco = scal.tile([P, 1], F32)
            nc.gpsimd.partition_broadcast(bco[:], go[:1, :], channels=P)
            s_o = scal.tile([P, 1], F32)
            nc.vector.tensor_scalar(s_o[:], bco[:], INV_QA, EPS, op0=ALU.mult, op1=ALU.max)
            inv_so = scal.tile([P, 1], F32)
            nc.vector.reciprocal(inv_so[:], s_o[:])
            # fp16 magic: out/s_o is in [-127,127], so 1536+v rounds to the
            # integer grid exactly in fp16 (ulp=1 in [1024,2048)).
            M16 = 1536.0
            m16_t = scal.tile([P, 1], F32)
            nc.vector.memset(m16_t[:], M16)
            nbt = scal.tile([P, 1], F32)  # -M16 * s_o for the ACT-second path
            nc.vector.tensor_scalar(nbt[:], s_o[:], -M16, None, op0=ALU.mult)

            # requant mostly on DVE (fp16 DVE ~4x faster than ACT); ACT takes
            # two of the eight pass1 ops. Output coalesced to 1MB DMAs via
            # idle wst ring slots.
            for pj in range(n_jt // 2):
                res = wst.tile([P, 2, TS], F16, tag="wst", name=f"ores{pj}")
                for h in range(2):
                    jt = 2 * pj + h
                    src = opre[:, jt, :]
                    oy = midp.tile([P, TS], F16, tag="mid")
                    if jt in (0, 4):
                        nc.scalar.activation(
                            oy[:], src, AF.Identity, bias=m16_t[:], scale=inv_so[:]
                        )
                    else:
                        nc.vector.tensor_scalar(
                            oy[:], src, inv_so[:], M16, op0=ALU.mult, op1=ALU.add
                        )
                    nc.vector.tensor_scalar(
                        res[:, h, :], oy[:], -M16, s_o[:], op0=ALU.add, op1=ALU.mult
                    )
                if pj == n_jt // 2 - 1:
                    nc.sync.dma_start(og_d[pj, :, 0:1, :], res[:, 0:1, :])
                    nc.scalar.dma_start(og_d[pj, :, 1:2, :], res[:, 1:2, :])
                else:
                    eng = nc.sync if pj % 2 == 0 else nc.scalar
                    eng.dma_start(og_d[pj, :, :, :], res[:])

    nc.compile()
    return nc


def _tile_pmajor(a2d, n_groups, gw):
    """[K, n_groups*gw] -> [n_groups, 128, K//128, gw] partition-major."""
    K = a2d.shape[0]
    return np.ascontiguousarray(
        a2d.reshape(K // 128, 128, n_groups, gw).transpose(2, 1, 0, 3)
    )


def _run(nc, inputs, n_cores, T, K, J, trace=False):
    from concourse.bass_utils import run_bass_kernel_spmd

    NTG, NJG = 2, 4
    TS, JS = T // NTG, J // NJG
    x = np.ascontiguousarray(inputs["x"], dtype=np.float32)
    w = np.ascontiguousarray(inputs["weight"], dtype=np.float32)
    b = np.ascontiguousarray(inputs["b"], dtype=np.float32)
    xT = np.ascontiguousarray(x.T.astype(np.float16))
    wT = np.ascontiguousarray(w.T)
    in_maps = []
    for c in range(n_cores):
        tg, jgr = divmod(c, NJG)
        xs = xT[:, tg * TS : (tg + 1) * TS]
        ws = wT[:, jgr * JS : (jgr + 1) * JS]
        bs = b[jgr * JS : (jgr + 1) * JS]
        # roll so the exclusive absmax sub-slice is the leading 512 columns
        xrr = np.roll(xs, -jgr * 512, axis=1)
        wrr = np.roll(ws, -tg * 512, axis=1)
        in_maps.append(
            {
                "xg": _tile_pmajor(xrr, TS // 512, 512),
                "wg": _tile_pmajor(wrr, JS // 512, 512),
                "b_full": b,
                "b_shard": np.ascontiguousarray(np.roll(bs, -tg * 512)),
            }
        )
    res = run_bass_kernel_spmd(nc, in_maps, core_ids=list(range(n_cores)), trace=trace)
    out = np.empty((T, J), dtype=np.float32)
    for c in range(n_cores):
        tg, jgr = divmod(c, NJG)
        og = res.results[c]["og"]  # [n_jt//2, 128, 2, TS]
        o = og.transpose(0, 2, 1, 3).reshape(JS, TS).astype(np.float32)
        o = np.roll(o, tg * 512, axis=0)
        o = np.roll(o, jgr * 512, axis=1)
        out[tg * TS : (tg + 1) * TS, jgr * JS : (jgr + 1) * JS] = o.T
    return out, res


_NC_CACHE = {}


def kernel(**inputs) -> np.ndarray:
    n_cores, T, K, J = 8, 4096, 4096, 4096
    key = (n_cores, T, K, J)
    if key not in _NC_CACHE:
        _NC_CACHE[key] = build(n_cores, T, K, J)
    out, _ = _run(_NC_CACHE[key], inputs, n_cores, T, K, J)
    return out


# revision 45
# speedup vs baseline: 1.2219x; 1.1510x over previous
"""Quantized Linear (8-bit act / 4-bit weight fake-quant) on 8 Trainium2 cores.

Math (per reference):
  xq = rne(x / s_x) * s_x          s_x = max(absmax(x)/127, 1e-8)
  wq = rne(w / s_w) * s_w          s_w = max(absmax(w)/7,   1e-8)
  bq = rne(b / s_b) * s_b          s_b = max(absmax(b)/127, 1e-8)
  out_pre = bq + xq @ wq.T
  out = rne(out_pre / s_o) * s_o   s_o = max(absmax(out_pre)/127, 1e-8)

v16 design (2 token-groups x 4 outf-groups over 8 cores), ~391us vs the
591us v6 baseline (479us re-measured). Matmul phase runs 99%+ PE-busy at
the P0-downclocked ~2.0GHz floor; remaining time is the absmax->scale
AllReduce prologue (pinned to ~95us by a ~75-80us CC-firmware startup
wall that no trigger timing can beat) and the out-scale mesh + requant
tail. The w re-read stream is gated on the PAIR-mesh readback (~80us,
after every core's absmax and mesh trigger are done) so the first w
tiles are resident when the scales land: matmul starts ~97.7us.
 - x cast to fp16 on host (halves x HBM traffic; integer quantization
   absorbs the rounding; measured rel-err 1.09e-2 < 2e-2, deterministic).
 - prologue: absmax slices read in 1-2MB chunks split across both HWDGE
   rings, landing INSIDE the qwT/qx ring slots (wabs/xabs; x-g0 then
   quantizes straight from xabs, never re-read). Chunked DVE reduces
   pipeline with the DMA. ONE combined [wmax,xmax] AllReduce instead of
   two serial meshes; a pairwise warmup mini-mesh absorbs the CC startup
   wall and aligns cores (removing it costs ~20us of mesh peer-wait).
 - bulk w/x re-read streams FIFO-gated behind a mesh-result readback on
   each ring: ungated prefetch steals shared HBM bandwidth from slower
   peers' absmax reads and inflates the mesh peer-wait (27us observed).
 - kt-outer / jt-inner matmul order into 8 PSUM banks: every quantized
   2-kt chunk immediately feeds 8 matmuls, so the PE chases the quantize
   chain without starving (99%+ busy window).
 - PSUM evictions (bank*s_xw + bq -> fp16 opre) alternate ACT/DVE per
   jt; out-absmax reductions chase the evictions on DVE. The last group's
   final 4 kt-rounds run pair-major so bank closes stagger ~2us apart and
   the eviction + out-absmax chain overlaps the matmul window (drain
   7.4us -> 3.5us; full jt-major serial chains would stall the PE).
 - tail: fp16-magic (1536) requant mostly on DVE (fp16 DVE ~4x ACT
   rate), results coalesced into 1MB fp16 DMAs through idle wst slots.
 - (tried and rejected: fp8 DoubleRow split-matmul (1.44x < 2x needed),
   remote_dma_broadcast scale exchange (delivery/slot mapping broken),
   jt-outer final group (serial bank chain stalls PE), 0.5MB absmax
   chunks (per-DMA fixed cost), all-DVE evictions (ACT idle while DVE
   drains).)
"""

import sys

sys.path.insert(0, "/opt/trn_rl_repo")

import numpy as np

import concourse.bass as bass
import concourse.mybir as mybir
import concourse.tile as tile
from concourse import bacc, bass_isa

F32 = mybir.dt.float32
F16 = mybir.dt.float16
BF16 = mybir.dt.bfloat16
AF = mybir.ActivationFunctionType
ALU = mybir.AluOpType
AX = mybir.AxisListType

MAGIC = 12582912.0  # 1.5 * 2**23: fp32 add rounds to nearest-even integer
EPS = 1e-8
INV_QA = float(np.float32(1.0) / np.float32(127.0))
INV_QW = float(np.float32(1.0) / np.float32(7.0))

P = 128


def build(n_cores=8, T=4096, K=4096, J=4096):
    """SPMD program; host rolls each core's columns so that the exclusive
    absmax sub-slices are always the local leading 512 columns."""
    NTG, NJG = 2, 4
    TS = T // NTG            # 2048 tokens per core
    JS = J // NJG            # 1024 out-features per core
    n_kp = K // P            # 32 k-tiles
    GT = 512                 # token group width
    n_g = TS // GT           # 4 token groups
    n_jt = JS // P           # 8 j-tiles
    SB = 2                   # k-tiles per chunk
    n_ch = n_kp // SB        # 16 chunks per 512-wide column group

    nc = bacc.Bacc(
        "TRN2", target_bir_lowering=False, debug=False, num_devices=n_cores,
        monotonic_sem_count=2,
    )

    xg_d = nc.dram_tensor("xg", [n_g, P, n_kp, GT], F16, kind="ExternalInput")
    wg_d = nc.dram_tensor("wg", [2, P, n_kp, 512], F32, kind="ExternalInput")
    b_d = nc.dram_tensor("b_full", [J], F32, kind="ExternalInput")
    bs_d = nc.dram_tensor("b_shard", [JS], F32, kind="ExternalInput")
    og_d = nc.dram_tensor("og", [n_jt // 2, P, 2, TS], F16, kind="ExternalOutput")
    cc1_in = nc.dram_tensor("cc1_in", [1, 2], F32)
    cc1_out = nc.dram_tensor("cc1_out", [1, 2], F32)
    ccp_in = nc.dram_tensor("ccp_in", [1, 1], F32)
    ccp_out = nc.dram_tensor("ccp_out", [1, 1], F32)
    cc2_in = nc.dram_tensor("cc2_in", [1, 1], F32)
    cc2_out = nc.dram_tensor("cc2_out", [1, 1], F32)
    groups = [list(range(n_cores))]
    rsem = nc.monotonic_semaphore(0)   # remote-recv counting (pinned num)
    lsem = nc.monotonic_semaphore(1)   # local send-complete (never waited)

    with tile.TileContext(nc) as tc:
        with (
            tc.tile_pool(name="const", bufs=1) as const,
            tc.tile_pool(name="scal", bufs=1) as scal,
            tc.tile_pool(name="wst", bufs=2) as wst,
            tc.tile_pool(name="xst", bufs=2) as xst,
            tc.tile_pool(name="mid", bufs=3) as midp,
            tc.tile_pool(name="wq", bufs=1) as wqp,
            tc.tile_pool(name="xq", bufs=2) as xqp,
            tc.tile_pool(name="op", bufs=1) as opp,
            tc.tile_pool(name="outst", bufs=2) as outst,
            tc.tile_pool(name="mm", bufs=8, space="PSUM") as mmps,
        ):
            magic_t = const.tile([P, 1], F32)
            nc.vector.memset(magic_t[:], MAGIC)
            nmagic_t = const.tile([P, 1], F32)
            nc.vector.memset(nmagic_t[:], -MAGIC)

            # ---- CC warmup: pairwise mini-mesh aligns core pairs early and
            # warms the CC engine so the tail out-mesh begins promptly ----
            nc.sync.dma_start(ccp_in[:], magic_t[:1, :])
            nc.gpsimd.collective_compute(
                "AllReduce", ALU.max,
                replica_groups=[[2 * i, 2 * i + 1] for i in range(n_cores // 2)],
                ins=[ccp_in[:]], outs=[ccp_out[:]],
            )

            # ---- tiny bias loads first (16KB; negligible ring delay) ----
            bfull = scal.tile([P, J // P], F32)
            nc.sync.dma_start(bfull[:], b_d.rearrange("(p a) -> p a", p=P))
            bsh = scal.tile([P, n_jt], F32)
            nc.sync.dma_start(bsh[:], bs_d.rearrange("(a p) -> p a", p=P))

            # ---------------- Phase A: exclusive-slice absmax ----------------
            # Big 2MB/1MB chunks (amortize per-DMA fixed cost), split across
            # both HWDGE rings. The landing tiles sit in the qwT / qx-slot-0
            # ring slots (zero extra SBUF); x-g0 later quantizes directly
            # from xabs, so x-g0 is never re-read.
            wabs = wqp.tile([P, n_kp, 512], F32, tag="wq", name="wabs")
            xabs = xqp.tile([P, n_kp, GT], F16, tag="qx", name="xabs")
            amw = scal.tile([P, 4], F32)
            amx = scal.tile([P, 4], F32)
            for q in range(4):
                weng = nc.scalar if q % 2 == 0 else nc.sync
                xeng = nc.sync if q % 2 == 0 else nc.scalar
                weng.dma_start(
                    wabs[:, q * 8 : (q + 1) * 8, :], wg_d[0, :, q * 8 : (q + 1) * 8, :]
                )
                nc.vector.tensor_reduce(
                    amw[:, q : q + 1], wabs[:, q * 8 : (q + 1) * 8, :],
                    axis=AX.XY, op=ALU.max, apply_absolute_value=True,
                )
                xeng.dma_start(
                    xabs[:, q * 8 : (q + 1) * 8, :], xg_d[0, :, q * 8 : (q + 1) * 8, :]
                )
                nc.vector.tensor_reduce(
                    amx[:, q : q + 1], xabs[:, q * 8 : (q + 1) * 8, :],
                    axis=AX.XY, op=ALU.max, apply_absolute_value=True,
                )

            m2 = scal.tile([P, 2], F32)
            nc.vector.tensor_reduce(m2[:, 0:1], amw[:], axis=AX.X, op=ALU.max)
            nc.vector.tensor_reduce(m2[:, 1:2], amx[:], axis=AX.X, op=ALU.max)
            # pre-scale by [1/7, 1/127] BEFORE the mesh (max commutes with
            # positive scaling exactly), so the post-mesh critical path is
            # 3 fused [P,2] ops instead of 7. Column writes via tensor_scalar
            # (memset on a column slice silently fails).
            m2s = scal.tile([P, 2], F32)
            nc.vector.tensor_scalar(m2s[:, 0:1], m2[:, 0:1], INV_QW, None, op0=ALU.mult)
            nc.vector.tensor_scalar(m2s[:, 1:2], m2[:, 1:2], INV_QA, None, op0=ALU.mult)
            g2 = scal.tile([P, 2], F32)
            nc.gpsimd.partition_all_reduce(
                g2[:], m2s[:], channels=P, reduce_op=bass_isa.ReduceOp.max
            )
            # cc input via the gpsimd SWDGE queue: par -> dma -> trigger all
            # FIFO on one engine, no cross-ring interference
            nc.gpsimd.dma_start(cc1_in[:], g2[:1, :])
            nc.gpsimd.collective_compute(
                "AllReduce", ALU.max, replica_groups=groups,
                ins=[cc1_in[:]], outs=[cc1_out[:]],
            )

            # ---- gates: bulk streams on both rings wait for the mesh ----
            # All 8 cores race their absmax reads against shared HBM
            # bandwidth; any bulk prefetch issued before the scales steals
            # bandwidth from slower peers' absmax reads. FIFO-ordering the
            # bulk DMAs behind a mesh-result readback keeps the race fair.
            gg = scal.tile([P, 2], F32)
            nc.sync.dma_start(gg[:1, :], cc1_out[:])
            # w-stream gate: the PAIR-mesh result, which lands ~12us before
            # the combined mesh completes. By then every core's absmax is
            # long done and every combined-mesh trigger has fired, so the
            # early w stream steals no critical bandwidth — but the first w
            # tiles are already in SBUF when the scales arrive, so quantize
            # (and the PE) starts immediately instead of waiting ~4us for
            # the first post-gate 1MB DMA.
            gpp = scal.tile([P, 1], F32)
            nc.scalar.dma_start(gpp[:1, :], ccp_out[:])

            # w: 1MB tiles of 4 k-tiles, all on the scalar ring.
            WCH = 4                      # k-tiles per w DMA tile
            n_wch = n_kp // WCH          # 8 tiles per 512-col group
            wre = []
            for i in range(2 * n_wch):
                jc, ii = divmod(i, n_wch)
                wf = wst.tile([P, WCH, 512], F32, tag="wst", name=f"wr{i}")
                nc.scalar.dma_start(
                    wf[:], wg_d[jc, :, ii * WCH : (ii + 1) * WCH, :]
                )
                wre.append(wf)

            # x: g1..g3 in 0.5MB tiles of 4 k-tiles on the sync ring
            # (g0 quantizes straight from xabs).
            xre = {}
            for g in range(1, n_g):
                for i in range(n_wch):
                    xf = xst.tile([P, WCH, GT], F16, tag="xst", name=f"xr{g}_{i}")
                    nc.sync.dma_start(xf[:], xg_d[g, :, i * WCH : (i + 1) * WCH, :])
                    xre[(g, i)] = xf

            bc2 = scal.tile([P, 2], F32)
            nc.gpsimd.partition_broadcast(bc2[:], gg[:1, :], channels=P)
            s2 = scal.tile([P, 2], F32)
            nc.vector.tensor_scalar(s2[:], bc2[:], EPS, None, op0=ALU.max)
            inv2 = scal.tile([P, 2], F32)
            nc.vector.reciprocal(inv2[:], s2[:])
            inv_sw = inv2[:, 0:1]
            inv_sx = inv2[:, 1:2]
            s_xw = scal.tile([P, 1], F32)
            nc.vector.tensor_tensor(out=s_xw[:], in0=s2[:, 1:2], in1=s2[:, 0:1], op=ALU.mult)

            # ---------------- Quantize helpers (ACT/DVE alternating) ---------
            def quant_chain(src_ap, dst_ap, inv_s, parity):
                mid = midp.tile([P, SB, 512], F32, tag="mid")
                if parity == 0:
                    nc.scalar.activation(
                        mid[:], src_ap, AF.Identity, bias=magic_t[:], scale=inv_s
                    )
                    nc.vector.tensor_scalar(dst_ap, mid[:], -MAGIC, None, op0=ALU.add)
                else:
                    nc.vector.tensor_scalar(
                        mid[:], src_ap, inv_s, MAGIC, op0=ALU.mult, op1=ALU.add
                    )
                    nc.scalar.activation(
                        dst_ap, mid[:], AF.Identity, bias=nmagic_t[:], scale=1.0
                    )

            # ---------------- W/X prep + matmul ------------------------------
            qwT = wqp.tile([P, n_kp, JS], BF16, tag="wq", name="qwT")

            def wsrc(i):  # i-th [P,SB,512] w chunk (0..31 across jc0,jc1)
                return wre[i // 2][:, (i % 2) * SB : (i % 2 + 1) * SB, :]

            def xsrc(g, i):  # i-th [P,SB,GT] x chunk of group g
                if g == 0:
                    return xabs[:, i * SB : (i + 1) * SB, :]
                return xre[(g, i // 2)][:, (i % 2) * SB : (i % 2 + 1) * SB, :]

            def wprep_jc(jc):
                for i in range(n_ch):
                    k0 = i * SB
                    quant_chain(
                        wsrc(jc * n_ch + i),
                        qwT[:, k0 : k0 + SB, jc * 512 : (jc + 1) * 512],
                        inv_sw, i % 2,
                    )

            def prep_group(g, qx=None):
                if qx is None:
                    qx = xqp.tile([P, n_kp, GT], BF16, tag="qx", name=f"qx_{g}")
                for i in range(n_ch):
                    k0 = i * SB
                    quant_chain(
                        xsrc(g, i), qx[:, k0 : k0 + SB, :], inv_sx, (i + 1) % 2
                    )
                return qx

            # jc0 + g0 interleaved so both streams advance together
            qx0 = xqp.tile([P, n_kp, GT], BF16, tag="qx", name="qx_0")
            for i in range(n_ch):
                k0 = i * SB
                quant_chain(
                    wsrc(i), qwT[:, k0 : k0 + SB, 0:512], inv_sw, i % 2
                )
                quant_chain(
                    xsrc(0, i), qx0[:, k0 : k0 + SB, :], inv_sx, (i + 1) % 2
                )

            # bias scale + quantized bias (local; needed by first eviction)
            bmax0 = scal.tile([P, 1], F32)
            nc.vector.tensor_reduce(
                bmax0[:], bfull[:], axis=AX.X, op=ALU.max,
                apply_absolute_value=True,
            )
            bmax = scal.tile([P, 1], F32)
            nc.gpsimd.partition_all_reduce(
                bmax[:], bmax0[:], channels=P, reduce_op=bass_isa.ReduceOp.max
            )
            s_b = scal.tile([P, 1], F32)
            nc.vector.tensor_scalar(s_b[:], bmax[:], INV_QA, EPS, op0=ALU.mult, op1=ALU.max)
            inv_sb = scal.tile([P, 1], F32)
            nc.vector.reciprocal(inv_sb[:], s_b[:])
            by = scal.tile([P, n_jt], F32)
            nc.scalar.activation(by[:], bsh[:], AF.Identity, bias=magic_t[:], scale=inv_sb[:])
            bq = scal.tile([P, n_jt], F32)
            nc.vector.tensor_scalar(bq[:], by[:], -MAGIC, s_b[:], op0=ALU.add, op1=ALU.mult)

            # rest of the quantize streams
            wprep_jc(1)
            qx1 = prep_group(1)

            opre = opp.tile([P, n_jt, TS], F16, tag="op", name="opre")
            omax = scal.tile([P, n_g * n_jt], F32)

            def mm_group(g, qx, stagger_tail=False):
                bks = []
                for jt in range(n_jt):
                    bks.append(mmps.tile([P, GT], F32, tag="mm", name=f"mm_{g}_{jt}"))

                def mm(kt, jt):
                    nc.tensor.matmul(
                        bks[jt][:],
                        lhsT=qwT[:, kt, jt * P : (jt + 1) * P],
                        rhs=qx[:, kt, :],
                        start=(kt == 0),
                        stop=(kt == n_kp - 1),
                    )

                n_head = n_kp - 4 if stagger_tail else n_kp
                for kt in range(n_head):
                    for jt in range(n_jt):
                        mm(kt, jt)
                if stagger_tail:
                    # last group only: run the final 4 kt-rounds pair-major so
                    # bank closes stagger ~2us apart and the per-jt eviction +
                    # out-absmax chain overlaps the matmul window instead of
                    # serializing ~7us after the final matmul. Pair-major
                    # keeps adjacent instructions on different banks (a full
                    # serial per-bank chain measurably stalls the PE).
                    for pair in range(n_jt // 2):
                        for kt in range(n_head, n_kp):
                            mm(kt, 2 * pair)
                            mm(kt, 2 * pair + 1)
                return bks

            def evict_group(g, bks):
                for jt in range(n_jt):
                    oc = opre[:, jt, g * GT : (g + 1) * GT]
                    if jt % 2 == 0:
                        nc.scalar.activation(
                            oc, bks[jt][:], AF.Identity,
                            bias=bq[:, jt : jt + 1], scale=s_xw[:],
                        )
                    else:
                        nc.vector.tensor_scalar(
                            oc, bks[jt][:], s_xw[:], bq[:, jt : jt + 1],
                            op0=ALU.mult, op1=ALU.add,
                        )
                    nc.vector.tensor_reduce(
                        omax[:, g * n_jt + jt : g * n_jt + jt + 1], oc,
                        axis=AX.X, op=ALU.max, apply_absolute_value=True,
                    )

            bks0 = mm_group(0, qx0)
            evict_group(0, bks0)
            qx2 = prep_group(2)
            qx3 = prep_group(3)
            bks1 = mm_group(1, qx1)
            evict_group(1, bks1)
            bks2 = mm_group(2, qx2)
            evict_group(2, bks2)
            bks3 = mm_group(3, qx3, stagger_tail=True)
            evict_group(3, bks3)

            # ---------------- Tail: global out absmax -> requantize ---------
            om1 = scal.tile([P, 1], F32)
            nc.vector.tensor_reduce(om1[:], omax[:], axis=AX.X, op=ALU.max)
            omr = scal.tile([P, 1], F32)
            nc.gpsimd.partition_all_reduce(
                omr[:], om1[:], channels=P, reduce_op=bass_isa.ReduceOp.max
            )
            nc.scalar.dma_start(cc2_in[:], omr[:1, :])
            nc.gpsimd.collective_compute(
                "AllReduce", ALU.max, replica_groups=groups,
                ins=[cc2_in[:]], outs=[cc2_out[:]],
            )
            go = scal.tile([P, 1], F32)
            nc.sync.dma_start(go[:1, :], cc2_out[:])
            bco = scal.tile([P, 1], F32)
            nc.gpsimd.partition_broadcast(bco[:], go[:1, :], channels=P)
            s_o = scal.tile([P, 1], F32)
            nc.vector.tensor_scalar(s_o[:], bco[:], INV_QA, EPS, op0=ALU.mult, op1=ALU.max)
            inv_so = scal.tile([P, 1], F32)
            nc.vector.reciprocal(inv_so[:], s_o[:])
            # fp16 magic: out/s_o is in [-127,127], so 1536+v rounds to the
            # integer grid exactly in fp16 (ulp=1 in [1024,2048)).
            M16 = 1536.0
            m16_t = scal.tile([P, 1], F32)
            nc.vector.memset(m16_t[:], M16)
            nbt = scal.tile([P, 1], F32)  # -M16 * s_o for the ACT-second path
            nc.vector.tensor_scalar(nbt[:], s_o[:], -M16, None, op0=ALU.mult)

            # requant mostly on DVE (fp16 DVE ~4x faster than ACT); ACT takes
            # two of the eight pass1 ops. Output coalesced to 1MB DMAs via
            # idle wst ring slots.
            # process all-DVE pairs (1,3) first, ACT-assisted pairs (0,2)
            # last: an ACT-fed pass2 at the DVE queue head would stall DVE
            # ~2.2us (ACT f16 ops run ~4x slower); this order hides the two
            # ACT pass1 ops entirely under DVE-only work.
            pj_order = [1, 3, 0, 2]
            for pi, pj in enumerate(pj_order):
                res = wst.tile([P, 2, TS], F16, tag="wst", name=f"ores{pj}")
                for h in range(2):
                    jt = 2 * pj + h
                    src = opre[:, jt, :]
                    oy = midp.tile([P, TS], F16, tag="mid")
                    if jt in (0, 4):
                        nc.scalar.activation(
                            oy[:], src, AF.Identity, bias=m16_t[:], scale=inv_so[:]
                        )
                    else:
                        nc.vector.tensor_scalar(
                            oy[:], src, inv_so[:], M16, op0=ALU.mult, op1=ALU.add
                        )
                    nc.vector.tensor_scalar(
                        res[:, h, :], oy[:], -M16, s_o[:], op0=ALU.add, op1=ALU.mult
                    )
                if pi == len(pj_order) - 1:
                    nc.sync.dma_start(og_d[pj, :, 0:1, :], res[:, 0:1, :])
                    nc.scalar.dma_start(og_d[pj, :, 1:2, :], res[:, 1:2, :])
                else:
                    eng = nc.sync if pi % 2 == 0 else nc.scalar
                    eng.dma_start(og_d[pj, :, :, :], res[:])

    nc.compile()
    return nc


def _tile_pmajor(a2d, n_groups, gw):
    """[K, n_groups*gw] -> [n_groups, 128, K//128, gw] partition-major."""
    K = a2d.shape[0]
    return np.ascontiguousarray(
        a2d.reshape(K // 128, 128, n_groups, gw).transpose(2, 1, 0, 3)
    )


def _run(nc, inputs, n_cores, T, K, J, trace=False):
    from concourse.bass_utils import run_bass_kernel_spmd

    NTG, NJG = 2, 4
    TS, JS = T // NTG, J // NJG
    x = np.ascontiguousarray(inputs["x"], dtype=np.float32)
    w = np.ascontiguousarray(inputs["weight"], dtype=np.float32)
    b = np.ascontiguousarray(inputs["b"], dtype=np.float32)
    xT = np.ascontiguousarray(x.T.astype(np.float16))
    wT = np.ascontiguousarray(w.T)
    in_maps = []
    for c in range(n_cores):
        tg, jgr = divmod(c, NJG)
        xs = xT[:, tg * TS : (tg + 1) * TS]
        ws = wT[:, jgr * JS : (jgr + 1) * JS]
        bs = b[jgr * JS : (jgr + 1) * JS]
        # roll so the exclusive absmax sub-slice is the leading 512 columns
        xrr = np.roll(xs, -jgr * 512, axis=1)
        wrr = np.roll(ws, -tg * 512, axis=1)
        in_maps.append(
            {
                "xg": _tile_pmajor(xrr, TS // 512, 512),
                "wg": _tile_pmajor(wrr, JS // 512, 512),
                "b_full": b,
                "b_shard": np.ascontiguousarray(np.roll(bs, -tg * 512)),
            }
        )
    res = run_bass_kernel_spmd(nc, in_maps, core_ids=list(range(n_cores)), trace=trace)
    out = np.empty((T, J), dtype=np.float32)
    for c in range(n_cores):
        tg, jgr = divmod(c, NJG)
        og = res.results[c]["og"]  # [n_jt//2, 128, 2, TS]
        o = og.transpose(0, 2, 1, 3).reshape(JS, TS).astype(np.float32)
        o = np.roll(o, tg * 512, axis=0)
        o = np.roll(o, jgr * 512, axis=1)
        out[tg * TS : (tg + 1) * TS, jgr * JS : (jgr + 1) * JS] = o.T
    return out, res


_NC_CACHE = {}


def kernel(**inputs) -> np.ndarray:
    n_cores, T, K, J = 8, 4096, 4096, 4096
    key = (n_cores, T, K, J)
    if key not in _NC_CACHE:
        _NC_CACHE[key] = build(n_cores, T, K, J)
    out, _ = _run(_NC_CACHE[key], inputs, n_cores, T, K, J)
    return out
